# revision 5
# baseline (speedup 1.0000x reference)
"""Trainium2 Bass kernel for nn_CADense (context-adaptive low-rank dense layer).

Computes, for the full batch:
    s_mod = s + context @ w          # [B, R]
    low   = (data @ u) * s_mod       # [B, R]
    out   = relu(low @ v.T + 2*bias) # [B, UNITS]

Sharding: data-parallel over batch across 8 NeuronCores; u/s/v/w/bias
replicated. Each core runs the same Bass program on its 1024-row shard.

The kernel sits at the bf16 ridge: PE streaming floor ~31us (73728
512-col matmul columns at 2.4GHz) vs ~33us of HBM traffic (7.25 MiB
loads + 4 MiB stores at ~353 GB/s/core). The schedule keeps both
saturated:

- All loads ride the sync HWDGE ring in exact first-consumption order,
  finely interleaved (u chunk g just before data chunk g) so the PE's
  rank stage is fed at its 296 GB/s consumption rate with no holes.
- The PE starts ~1.6us in on a few garbage warm-up matmuls (reading
  uninitialized SBUF - results are never read) purely to open the HAM
  p-state ramp; the ~3us 1.2GHz ramp window then does REAL work
  (smod + first rank chunks), which is load-paced anyway.
- Bulk output stores are queued on the SAME sync ring AFTER all loads:
  the SP sequencer's FIFO order guarantees loads get all of the HBM
  bandwidth until they finish (~22us), then the accumulated store
  backlog drains at full rate behind them.
- The final two output groups evacuate and store per-m-chunk on the
  scalar/gpsimd/sync rings so the last 128KB pieces land with minimal
  latency after the last matmul.
- outT stays transposed so the 2*bias term is a per-partition scalar
  fused into PSUM evacuation (scalar.activation bias / DVE max+add).
"""

import os
import sys
from contextlib import ExitStack

import numpy as np
import ml_dtypes


def _ensure_concourse():
    try:
        import concourse  # noqa: F401
    except ImportError:
        for p in ("/opt/trn_rl_repo", "/root/.axon_site/_ro/trn_rl_repo"):
            if os.path.isdir(p) and p not in sys.path:
                sys.path.insert(0, p)


_ensure_concourse()

import concourse.tile as tile  # noqa: E402
from concourse import bacc, mybir  # noqa: E402
from concourse.bass_utils import run_bass_kernel_spmd  # noqa: E402

NCORES = 8
B, N_IN, UNITS, RANK, CCTX = 8192, 2048, 2048, 256, 512
NB = B // NCORES  # batch rows per core
P = 128
BT = 512  # batch tile (free dim of matmuls)
NBT = NB // BT  # 2 batch tiles per core
KC = N_IN // P  # 16 contraction chunks for data @ u
CC = CCTX // P  # 4 contraction chunks for context @ w
RC = RANK // P  # 2 rank chunks
MC = UNITS // P  # 16 output unit chunks (partition dim of outT)
N_WARMUP_MM = 10  # 128-col garbage matmuls bridging until the first real work

F32 = mybir.dt.float32
BF16 = mybir.dt.bfloat16
NP_BF16 = np.dtype(ml_dtypes.bfloat16)


def _emit(nc, tc, ctx):
    d_dataT = nc.dram_tensor("dataT", [N_IN, NB], BF16, kind="ExternalInput")
    d_ctxT = nc.dram_tensor("ctxT", [CCTX, NB], BF16, kind="ExternalInput")
    d_u = nc.dram_tensor("u", [N_IN, RANK], BF16, kind="ExternalInput")
    d_s = nc.dram_tensor("s", [RANK], F32, kind="ExternalInput")
    d_vT = nc.dram_tensor("vT", [RANK, UNITS], BF16, kind="ExternalInput")
    d_w = nc.dram_tensor("w", [CCTX, RANK], BF16, kind="ExternalInput")
    d_bias = nc.dram_tensor("bias", [UNITS], F32, kind="ExternalInput")
    d_outT = nc.dram_tensor("outT", [UNITS, NB], BF16, kind="ExternalOutput")

    ap_dataT = d_dataT.ap().rearrange("(c p) b -> p c b", p=P)
    ap_ctxT = d_ctxT.ap().rearrange("(cc p) b -> p cc b", p=P)
    ap_u = d_u.ap().rearrange("(q p) r -> p q r", p=P)
    ap_w = d_w.ap().rearrange("(cc p) r -> p cc r", p=P)
    ap_vT = d_vT.ap().rearrange("(rc p) m -> p rc m", p=P)
    ap_outT = d_outT.ap().rearrange("(mc p) b -> p mc b", p=P)

    singles = ctx.enter_context(tc.tile_pool(name="singles", bufs=1))
    wu_psum = ctx.enter_context(tc.tile_pool(name="wu_psum", bufs=1, space="PSUM"))
    du_psum = ctx.enter_context(tc.tile_pool(name="du_psum", bufs=2, space="PSUM"))
    s_psum = ctx.enter_context(tc.tile_pool(name="s_psum", bufs=2, space="PSUM"))
    o_psum = ctx.enter_context(tc.tile_pool(name="o_psum", bufs=3, space="PSUM"))

    # Warm-up operand: one small tile, memset on the (otherwise idle)
    # gpsimd engine so the first LDWEIGHTS can issue right after the entry
    # barrier, opening the HAM p-state ramp early.
    wu_a = singles.tile([P, P], BF16, name="wu_a")
    nc.gpsimd.memset(wu_a[:], 1.0)

    # ---- SBUF tiles ----------------------------------------------------
    u_t = {g: singles.tile([P, 4, RANK], BF16, name=f"u{g}") for g in range(4)}
    d_t = {}
    # data(bt0) first group split in 2-kc halves for a finer first arrival.
    D0_CHUNKS = [(0, 2), (2, 2), (4, 4), (8, 4), (12, 4)]  # (kc_lo, width)
    for g, (kc_lo, w_kc) in enumerate(D0_CHUNKS):
        d_t[(0, g)] = singles.tile([P, w_kc, BT], BF16, name=f"d0g{g}")
    for g in range(4):
        d_t[(1, g)] = singles.tile([P, 4, BT], BF16, name=f"d1g{g}")

    def data_chunk(bt, kc):
        if bt == 0:
            for g, (kc_lo, w_kc) in enumerate(D0_CHUNKS):
                if kc_lo <= kc < kc_lo + w_kc:
                    return d_t[(0, g)][:, kc - kc_lo, :]
        return d_t[(1, kc // 4)][:, kc % 4, :]

    s_sb = singles.tile([P, RC], F32, name="s_sb")
    bias_sb = singles.tile([P, MC], F32, name="bias_sb")
    bias2 = singles.tile([P, MC], F32, name="bias2")
    nbias2 = singles.tile([P, MC], F32, name="nbias2")
    w_sb = singles.tile([P, CC, RANK], BF16, name="w_sb")
    ctx0_t = {h: singles.tile([P, 2, BT], BF16, name=f"ctx0h{h}") for h in range(2)}
    ctx1 = singles.tile([P, CC, BT], BF16, name="ctx1")
    # vT split by mc-halves so the first out groups don't wait on the tail
    # of a monolithic 1 MiB load.
    vT_sb = singles.tile([P, RC, UNITS], BF16, name="vT_sb")

    def ctx_chunk(bt, cc):
        if bt == 0:
            return ctx0_t[cc // 2][:, cc % 2, :]
        return ctx1[:, cc, :]

    smod = singles.tile([P, RC, NB], F32, name="smod")
    lowT = {
        (bt, rc): singles.tile([P, BT], BF16, name=f"lowT{bt}r{rc}")
        for bt in range(NBT)
        for rc in range(RC)
    }
    osb = {
        (bt, g): singles.tile([P, 4, BT], BF16, name=f"o{bt}g{g}")
        for bt in range(NBT)
        for g in range(4)
    }

    # ---- load queue (sync HWDGE ring), exact first-consumption order ----
    def ld(out, in_):
        nc.sync.dma_start(out=out, in_=in_)

    def load_data(bt, g):
        if bt == 0:
            kc_lo, w_kc = D0_CHUNKS[g]
        else:
            kc_lo, w_kc = 4 * g, 4
        ld(d_t[(bt, g)][:], ap_dataT[:, kc_lo : kc_lo + w_kc, bt * BT : (bt + 1) * BT])

    ld(ctx0_t[0][:], ap_ctxT[:, 0:2, 0:BT])
    ld(w_sb[:], ap_w)
    ld(u_t[0][:], ap_u[:, 0:4])
    load_data(0, 0)  # kc0-1
    ld(ctx0_t[1][:], ap_ctxT[:, 2:4, 0:BT])
    load_data(0, 1)  # kc2-3
    ld(s_sb[:], d_s.ap().rearrange("(rc p) -> p rc", p=P))
    ld(bias_sb[:], d_bias.ap().rearrange("(mc p) -> p mc", p=P))
    ld(u_t[1][:], ap_u[:, 4:8])
    load_data(0, 2)  # kc4-7
    ld(ctx1[:], ap_ctxT[:, :, BT:])
    ld(u_t[2][:], ap_u[:, 8:12])
    load_data(0, 3)  # kc8-11
    ld(u_t[3][:], ap_u[:, 12:16])
    load_data(0, 4)  # kc12-15
    ld(vT_sb[:, :, 0:1024], ap_vT[:, :, 0:1024])  # mc0-7, both rc
    ld(vT_sb[:, :, 1024:2048], ap_vT[:, :, 1024:2048])  # mc8-15
    load_data(1, 0)
    load_data(1, 1)
    load_data(1, 2)
    load_data(1, 3)

    # ---- compute stages ------------------------------------------------
    wu_ps = wu_psum.tile([P, BT], F32, tag="wu", name="wu_ps")

    def emit_warmups(n):
        for _ in range(n):
            nc.tensor.matmul(
                wu_ps[:, 0:P], lhsT=wu_a[:], rhs=wu_a[:], start=True, stop=True
            )

    pd_t = {}
    ps_t = {}

    def emit_smod_mms(bt, cc_lo, cc_hi):
        """ctx @ w matmuls, cc-outer so they chase the ctx halves."""
        if cc_lo == 0:
            ps_t[bt] = [s_psum.tile([P, BT], F32, tag="ps", name="ps") for _ in range(RC)]
        for cc in range(cc_lo, cc_hi):
            for rc in range(RC):
                nc.tensor.matmul(
                    ps_t[bt][rc][:],
                    lhsT=w_sb[:, cc, rc * P : (rc + 1) * P],
                    rhs=ctx_chunk(bt, cc),
                    start=(cc == 0),
                    stop=(cc == CC - 1),
                )

    def emit_smod_evac(bt):
        for rc in range(RC):
            nc.scalar.add(
                smod[:, rc, bt * BT : (bt + 1) * BT],
                ps_t[bt][rc][:],
                add=s_sb[:, rc : rc + 1],
            )

    def emit_rank_mms(bt, kc_lo, kc_hi):
        if kc_lo == 0:
            pd_t[bt] = [du_psum.tile([P, BT], F32, tag="pd", name="pd") for _ in range(RC)]
        for kc in range(kc_lo, kc_hi):
            for rc in range(RC):
                nc.tensor.matmul(
                    pd_t[bt][rc][:],
                    lhsT=u_t[kc // 4][:, kc % 4, rc * P : (rc + 1) * P],
                    rhs=data_chunk(bt, kc),
                    start=(kc == 0),
                    stop=(kc == KC - 1),
                )

    def emit_mul(bt):
        """lowT = pd * smod on the vector engine (bf16 out)."""
        for rc in range(RC):
            nc.vector.tensor_mul(
                out=lowT[(bt, rc)][:],
                in0=pd_t[bt][rc][:],
                in1=smod[:, rc, bt * BT : (bt + 1) * BT],
            )

    def emit_out_group(bt, g, fine_stores=None):
        """outT[m, b] = relu(vT.T @ lowT + 2*bias) for 4 m-chunks.

        fine_stores: None -> one coarse store for the group on the sync
        ring (FIFO behind the loads); else a list of 4 engines for
        per-m-chunk stores.
        """
        ob = osb[(bt, g)]
        for j in range(4):
            mc = 4 * g + j
            if bt == 1:
                pool = (o_psum, s_psum, o_psum, du_psum)[j]
                tag = ("po", "ps", "po", "pd")[j]
            else:
                pool = (o_psum, s_psum, o_psum, o_psum)[j]
                tag = ("po", "ps", "po", "po")[j]
            po = pool.tile([P, BT], F32, tag=tag, name="po")
            for rc in range(RC):
                nc.tensor.matmul(
                    po[:],
                    lhsT=vT_sb[:, rc, mc * P : (mc + 1) * P],
                    rhs=lowT[(bt, rc)][:],
                    start=(rc == 0),
                    stop=(rc == RC - 1),
                )
            if mc % 2 == 0:
                nc.scalar.activation(
                    ob[:, j, :],
                    po[:],
                    mybir.ActivationFunctionType.Relu,
                    bias=bias2[:, mc : mc + 1],
                )
            else:
                nc.vector.tensor_scalar(
                    out=ob[:, j, :],
                    in0=po[:],
                    scalar1=nbias2[:, mc : mc + 1],
                    scalar2=bias2[:, mc : mc + 1],
                    op0=mybir.AluOpType.max,
                    op1=mybir.AluOpType.add,
                )
            if fine_stores is not None:
                fine_stores[j].dma_start(
                    out=ap_outT[:, mc, bt * BT : (bt + 1) * BT], in_=ob[:, j, :]
                )

    def store_group(bt, g):
        nc.sync.dma_start(
            out=ap_outT[:, 4 * g : 4 * g + 4, bt * BT : (bt + 1) * BT],
            in_=osb[(bt, g)][:],
        )

    # ---- software pipeline, PE emission in DMA-arrival order -----------
    emit_warmups(N_WARMUP_MM)
    emit_smod_mms(0, 0, 2)  # ctx0h0 + w
    emit_rank_mms(0, 0, 2)  # u0, d0 kc0-1
    emit_smod_mms(0, 2, 4)  # ctx0h1
    emit_smod_evac(0)
    nc.scalar.mul(bias2[:], bias_sb[:], 2.0)
    nc.scalar.mul(nbias2[:], bias_sb[:], -2.0)
    emit_rank_mms(0, 2, 4)  # d0 kc2-3
    emit_rank_mms(0, 4, 8)  # u1, d0 kc4-7
    emit_smod_mms(1, 0, 4)  # ctx1
    emit_smod_evac(1)
    emit_rank_mms(0, 8, 12)  # u2, d0 kc8-11
    emit_rank_mms(0, 12, 16)  # u3, d0 kc12-15
    emit_mul(0)
    emit_out_group(0, 0)  # vT mc0-7 half
    emit_rank_mms(1, 0, 4)
    emit_out_group(0, 1)
    emit_rank_mms(1, 4, 8)
    emit_out_group(0, 2)  # vT mc8-15 half
    emit_rank_mms(1, 8, 12)
    emit_out_group(0, 3)
    emit_rank_mms(1, 12, 16)
    emit_mul(1)
    emit_out_group(1, 0)
    emit_out_group(1, 1)
    emit_out_group(1, 2, fine_stores=[nc.scalar, nc.gpsimd, nc.scalar, nc.gpsimd])
    emit_out_group(1, 3, fine_stores=[nc.gpsimd, nc.scalar, nc.sync, nc.gpsimd])
    # Bulk stores: FIFO on the sync ring behind every load, so loads keep
    # the full HBM bandwidth until they finish, then the store backlog
    # drains at full rate.
    store_group(0, 0)
    store_group(0, 1)
    store_group(0, 2)
    store_group(0, 3)
    store_group(1, 0)
    store_group(1, 1)


_CACHE = {}


def build():
    if "nc" in _CACHE:
        return _CACHE["nc"]
    nc = bacc.Bacc("TRN2", target_bir_lowering=False, debug=False)
    with tile.TileContext(nc) as tc, ExitStack() as ctx:
        _emit(nc, tc, ctx)
    nc.compile()
    _CACHE["nc"] = nc
    return nc


def make_in_maps(data, context, u, s, v, w, bias):
    u_b = np.ascontiguousarray(np.asarray(u, dtype=np.float32)).astype(NP_BF16)
    s = np.ascontiguousarray(np.asarray(s, dtype=np.float32))
    vT_b = np.ascontiguousarray(np.asarray(v, dtype=np.float32).T).astype(NP_BF16)
    w_b = np.ascontiguousarray(np.asarray(w, dtype=np.float32)).astype(NP_BF16)
    bias = np.ascontiguousarray(np.asarray(bias, dtype=np.float32))
    data = np.asarray(data, dtype=np.float32)
    context = np.asarray(context, dtype=np.float32)
    in_maps = []
    for c in range(NCORES):
        sl = slice(c * NB, (c + 1) * NB)
        in_maps.append(
            {
                "dataT": np.ascontiguousarray(data[sl].T).astype(NP_BF16),
                "ctxT": np.ascontiguousarray(context[sl].T).astype(NP_BF16),
                "u": u_b,
                "s": s,
                "vT": vT_b,
                "w": w_b,
                "bias": bias,
            }
        )
    return in_maps


def kernel(data, context, u, s, v, w, bias):
    nc = build()
    in_maps = make_in_maps(data, context, u, s, v, w, bias)
    res = run_bass_kernel_spmd(nc, in_maps, core_ids=list(range(NCORES)))
    return np.concatenate(
        [np.asarray(r["outT"]).astype(np.float32).T for r in res.results], axis=0
    )
